# revision 15
# baseline (speedup 1.0000x reference)
"""Trainium2 Bass kernel for nn_ComplementaryContrastiveLearning.

Reference computation (B=8192, D=512, fp32):
    noisy        = confident_noisy_mask != 0
    negative_mask[i,j] = noisy[i] | noisy[j]                       (as f32)
    vn, tn       = l2_normalize(visual), l2_normalize(textual)
    sims         = vn @ tn.T                                       [B, B]
    loss         = mean(log(sum(exp(sims/T) * negative_mask, 1) + EPS))
    returns (loss, loss, negative_mask, sims)

Sharding: rows of visual (and of sims / negative_mask) split across 8 cores,
textual replicated.  Each core:
  - normalizes its visual rows and all textual rows on-device
    (inv_norm = exp(-0.5 * ln(sum(x^2))) so the scalar engine only ever needs
    the natural_log_exp_and_others table set - no table-switch thrash),
  - transposes both to feature-major via PE transposes,
  - computes the [1024, 8192] sims block on the PE (PSUM accumulation over 4
    K=128 chunks), copies PSUM -> SBUF -> DRAM,
  - folds the negative mask into the logits with one extra K=1 matmul:
    PSUM += -b_i*b_j with b_i = sqrt(BIG*T)*(1-noisy_i), so
    exp(PSUM/T) = exp(sims/T - BIG*(1-n_i)(1-n_j)): masked-out entries get
    a factor e^-30 ~ 1e-13 (negligible vs row sums ~1e3..1e6),
  - one ACT pass computes exp(PSUM/T) with a fused row-sum (accum_out),
  - log(rowsum + EPS), per-core partial sum of logs via a ones-matmul
    partition reduction; host sums partials / B.
The mask output is built from a host-broadcast [128, B] noisy row with a
per-partition tensor_scalar max, one row-block at a time.
"""

import numpy as np
from contextlib import ExitStack

import concourse.bass as bass
import concourse.tile as tile
from concourse import mybir
from concourse.bass_utils import run_bass_kernel_spmd

F32 = mybir.dt.float32

B = 8192
D = 512
N_CORES = 8
ROWS = B // N_CORES          # 1024 visual rows per core
P = 128                      # partitions
TEMP = 0.07
EPS = 1e-8
BIG = 30.0
BV = float(np.sqrt(np.float64(BIG) * np.float64(TEMP)))  # b_i scale

# matmul input interpretation: float32r = 1 cyc/row (fp22 truncated),
# float32 = 4 cyc/row (exact fp32). Flip here to trade accuracy vs speed.
MM_DT = mybir.dt.float32r


def _build(rows=ROWS, b_total=B, mm_dt=MM_DT, n_tile=512, legalize=True):
    """Build the per-core Bass program (SPMD; every core runs the same code
    on its own row slice)."""
    m_blocks = rows // P              # row blocks of 128
    n_slices = b_total // n_tile      # column slices of n_tile
    c_chunks = D // P                 # 4 contraction chunks of 128

    nc = bass.Bass("TRN2", target_bir_lowering=False, debug=False)

    visual = nc.dram_tensor("visual", [rows, D], F32, kind="ExternalInput").ap()
    textual = nc.dram_tensor("textual", [b_total, D], F32, kind="ExternalInput").ap()
    bnoisy = nc.dram_tensor("bnoisy", [P, b_total], F32, kind="ExternalInput").ap()
    noisyv = nc.dram_tensor("noisyv", [P, m_blocks], F32, kind="ExternalInput").ap()
    negbv = nc.dram_tensor("negbv", [1, rows], mm_dt, kind="ExternalInput").ap()
    bt = nc.dram_tensor("bt", [1, b_total], mm_dt, kind="ExternalInput").ap()

    sims = nc.dram_tensor("sims", [rows, b_total], F32, kind="ExternalOutput").ap()
    mask = nc.dram_tensor("mask", [rows, b_total], F32, kind="ExternalOutput").ap()
    partial = nc.dram_tensor("partial", [1, 1], F32, kind="ExternalOutput").ap()

    ident_h = nc.inline_tensor(np.eye(P, dtype=np.float32), name="ident")

    AF = mybir.ActivationFunctionType
    ALU = mybir.AluOpType
    AX = mybir.AxisListType

    with tile.TileContext(nc) as tc:
        with ExitStack() as ctx:
            const_pool = ctx.enter_context(tc.tile_pool(name="const", bufs=1))
            small_pool = ctx.enter_context(tc.tile_pool(name="small", bufs=4))
            prep_pool = ctx.enter_context(tc.tile_pool(name="prep", bufs=4))
            tT_pool = ctx.enter_context(tc.tile_pool(name="tT", bufs=2))
            sims_pool = ctx.enter_context(tc.tile_pool(name="simsb", bufs=4))
            mask_pool = ctx.enter_context(tc.tile_pool(name="maskb", bufs=2))
            stat_pool = ctx.enter_context(tc.tile_pool(name="stats", bufs=1))
            scr_pool = ctx.enter_context(tc.tile_pool(name="scr", bufs=2))
            ps_acc = ctx.enter_context(tc.tile_pool(name="ps", bufs=3, space="PSUM"))
            ps_tr = ctx.enter_context(tc.tile_pool(name="pst", bufs=2, space="PSUM"))

            # ---- constants / persistent tiles ----
            ident_t = const_pool.tile([P, P], F32, tag="ident")
            nc.sync.dma_start(ident_t[:], ident_h.ap()[:, :])
            ones_t = const_pool.tile([P, 1], F32, tag="ones")
            nc.vector.memset(ones_t[:], 1.0)
            eps_t = const_pool.tile([P, 1], F32, tag="eps")
            nc.vector.memset(eps_t[:], EPS)
            bnoisy_t = const_pool.tile([P, b_total], F32, tag="bnoisy")
            nc.sync.dma_start(bnoisy_t[:], bnoisy[:, :])
            noisyv_t = const_pool.tile([P, m_blocks], F32, tag="noisyv")
            nc.sync.dma_start(noisyv_t[:], noisyv[:, :])
            negbv_t = const_pool.tile([1, rows], mm_dt, tag="negbv")
            nc.sync.dma_start(negbv_t[:], negbv[:, :])
            bt_t = const_pool.tile([1, b_total], mm_dt, tag="bt")
            nc.sync.dma_start(bt_t[:], bt[:, :])

            # Absorb the const-DMA completions into the DVE engine clock up
            # front: walrus allows only a few sync-wait commands per
            # instruction, and the first consumer of several const tiles
            # would otherwise collect one wait per DMA lane.
            join_t = small_pool.tile([1, 1], F32, tag="join")
            nc.vector.tensor_copy(join_t[:], bnoisy_t[0:1, 0:1])
            nc.vector.tensor_copy(join_t[:], noisyv_t[0:1, 0:1])

            # visualT, normalized, feature-major: [feat_in_chunk, chunk, row]
            vT_t = const_pool.tile([P, c_chunks, rows], mm_dt, tag="vT")
            # per-row-block running stats: stats[m][:, n] = masked rowsum part
            n_units = b_total // (2 * n_tile)
            stats = [
                stat_pool.tile([P, n_units], F32, tag=f"st{m}", name=f"st{m}")
                for m in range(m_blocks)
            ]
            losscol = stat_pool.tile([P, m_blocks], F32, tag="losscol")

            def normalize_tile(src_ap, tag_prefix):
                """Load [128, D] rows, return SBUF tile of l2-normalized rows."""
                raw = prep_pool.tile([P, D], F32, tag=f"{tag_prefix}raw")
                nc.sync.dma_start(raw[:], src_ap)
                sq = scr_pool.tile([P, D], F32, tag="scr")
                ss = small_pool.tile([P, 1], F32, tag="ss")
                nc.scalar.activation(sq[:], raw[:], AF.Square, accum_out=ss[:])
                lns = small_pool.tile([P, 1], F32, tag="lns")
                nc.scalar.activation(lns[:], ss[:], AF.Ln)
                inv = small_pool.tile([P, 1], F32, tag="inv")
                nc.scalar.activation(inv[:], lns[:], AF.Exp, scale=-0.5)
                norm = prep_pool.tile([P, D], F32, tag=f"{tag_prefix}norm")
                nc.vector.tensor_scalar_mul(norm[:], raw[:], inv[:])
                return norm

            def transpose_to(dst_ap, norm_tile):
                """PE-transpose [128, D] -> [feat, chunk, 128 rows] into dst."""
                pst = ps_tr.tile([P, c_chunks, P], F32, tag="pst")
                for c in range(c_chunks):
                    nc.tensor.transpose(
                        pst[:, c, :], norm_tile[:, c * P:(c + 1) * P], ident_t[:]
                    )
                nc.vector.tensor_copy(dst_ap, pst[:])

            # ---- prologue: normalize + transpose the visual chunk ----
            for m in range(m_blocks):
                vn = normalize_tile(visual[m * P:(m + 1) * P, :], "v")
                transpose_to(vT_t[:, :, m * P:(m + 1) * P], vn)

            # ---- negative-mask output (independent of the matmul path) ----
            mchunk = min(2048, b_total)
            for m in range(m_blocks):
                for j in range(0, b_total, mchunk):
                    mb_t = mask_pool.tile([P, mchunk], F32, tag="maskblk")
                    nc.gpsimd.tensor_scalar(
                        out=mb_t[:],
                        in0=bnoisy_t[:, j:j + mchunk],
                        scalar1=noisyv_t[:, m:m + 1],
                        scalar2=None,
                        op0=ALU.max,
                    )
                    nc.sync.dma_start(
                        mask[m * P:(m + 1) * P, j:j + mchunk], mb_t[:])

            # ---- main loop: textual prep + matmul + (delayed) fold/exp ----
            # Each unit is a [128, 2*n_tile] PSUM accumulator (2 banks).  The
            # mask-fold K=1 matmul + exp of unit u-1 are emitted after unit
            # u's main matmuls so the PE never stalls waiting on the sims
            # copy of the unit it just produced (keeps PE dense -> HAM warm).
            unit = 2 * n_tile
            pending = None

            def fold_and_exp(ps, m, u):
                for h in range(2):
                    j0 = u * unit + h * n_tile
                    nc.tensor.matmul(
                        ps[:, h * n_tile:(h + 1) * n_tile],
                        lhsT=negbv_t[0:1, m * P:(m + 1) * P],
                        rhs=bt_t[0:1, j0:j0 + n_tile],
                        start=False,
                        stop=True,
                        skip_group_check=True,
                    )
                nc.scalar.activation(
                    ps[:], ps[:], AF.Exp,
                    scale=1.0 / TEMP,
                    accum_out=stats[m][:, u:u + 1],
                )

            for u in range(n_units):
                tTn = tT_pool.tile([P, c_chunks, unit], mm_dt, tag="tTn")
                for t in range(unit // P):
                    row0 = u * unit + t * P
                    tn_tile = normalize_tile(textual[row0:row0 + P, :], "t")
                    transpose_to(tTn[:, :, t * P:(t + 1) * P], tn_tile)

                for m in range(m_blocks):
                    ps = ps_acc.tile([P, unit], F32, tag="acc")
                    for h in range(2):
                        for c in range(c_chunks):
                            nc.tensor.matmul(
                                ps[:, h * n_tile:(h + 1) * n_tile],
                                lhsT=vT_t[:, c, m * P:(m + 1) * P],
                                rhs=tTn[:, c, h * n_tile:(h + 1) * n_tile],
                                start=(c == 0),
                                stop=(c == c_chunks - 1),
                            )
                    stile = sims_pool.tile([P, unit], F32, tag="stile")
                    nc.vector.tensor_copy(stile[:], ps[:])
                    nc.sync.dma_start(
                        sims[m * P:(m + 1) * P, u * unit:(u + 1) * unit],
                        stile[:],
                    )
                    if pending is not None:
                        fold_and_exp(*pending)
                    pending = (ps, m, u)
            fold_and_exp(*pending)

            # ---- epilogue: log(rowsum + eps), partition-sum, partial out ----
            for m in range(m_blocks):
                rowsum = small_pool.tile([P, 1], F32, tag="rowsum")
                nc.vector.reduce_sum(rowsum[:], stats[m][:], axis=AX.X)
                nc.scalar.activation(
                    losscol[:, m:m + 1], rowsum[:], AF.Ln, bias=eps_t[:]
                )
            total = small_pool.tile([P, 1], F32, tag="total")
            nc.vector.reduce_sum(total[:], losscol[:], axis=AX.X)
            ps1 = ps_acc.tile([1, 1], F32, tag="acc")
            nc.tensor.matmul(ps1[:], lhsT=ones_t[:], rhs=total[:],
                             start=True, stop=True)
            out1 = small_pool.tile([1, 1], F32, tag="out1")
            nc.vector.tensor_copy(out1[:], ps1[:])
            nc.sync.dma_start(partial[:, :], out1[:])

    return _legalize_waits(nc) if legalize else nc


def _legalize_waits(nc):
    """This toolchain's walrus codegen accepts at most ONE sync-wait command
    per instruction; the Tile scheduler freely attaches several.  Hoist the
    extras onto standalone EventSemaphore instructions (same engine, directly
    before the instruction) - program order on the engine preserves the gate
    semantics exactly."""
    for func in nc.m.functions:
        for block in func.blocks:
            out = []
            for ins in block.instructions:
                si = ins.sync_info
                waits = list(si.on_wait) if si is not None and si.on_wait else []
                if len(waits) > 1:
                    for k, w in enumerate(waits[:-1]):
                        ev = mybir.InstEventSemaphore(name=f"{ins.name}-ew{k}")
                        ev.engine = ins.engine
                        ev.sync_info = mybir.SyncInfo(on_wait=[w], on_update=[])
                        out.append(ev)
                    ins.sync_info = mybir.SyncInfo(
                        on_wait=[waits[-1]],
                        on_update=list(si.on_update) if si.on_update else [],
                    )
                out.append(ins)
            if len(out) != len(block.instructions):
                block.instructions = out
    return nc


_NC_CACHE = {}


def _get_nc():
    key = (ROWS, B, MM_DT)
    if key not in _NC_CACHE:
        _NC_CACHE[key] = _build()
    return _NC_CACHE[key]


def _host_inputs(visual_feats, textual_feats, confident_noisy_mask):
    vis = np.ascontiguousarray(np.asarray(visual_feats, dtype=np.float32))
    tex = np.ascontiguousarray(np.asarray(textual_feats, dtype=np.float32))
    noisy_f = (np.asarray(confident_noisy_mask) != 0).astype(np.float32)
    b_vec = (BV * (1.0 - noisy_f)).astype(np.float32)          # [B]
    bnoisy = np.ascontiguousarray(np.broadcast_to(noisy_f, (P, B)))
    bt_2d = np.ascontiguousarray(b_vec.reshape(1, B))
    m_blocks = ROWS // P
    in_maps = []
    for c in range(N_CORES):
        rs = slice(c * ROWS, (c + 1) * ROWS)
        in_maps.append({
            "visual": vis[rs],
            "textual": tex,
            "bnoisy": bnoisy,
            "noisyv": np.ascontiguousarray(
                noisy_f[rs].reshape(m_blocks, P).T),
            "negbv": np.ascontiguousarray((-b_vec[rs]).reshape(1, ROWS)),
            "bt": bt_2d,
        })
    return in_maps


def kernel(visual_feats, textual_feats, confident_clean_mask,
           confident_noisy_mask, _trace=False):
    nc = _get_nc()
    in_maps = _host_inputs(visual_feats, textual_feats, confident_noisy_mask)
    res = run_bass_kernel_spmd(nc, in_maps, list(range(N_CORES)), trace=_trace)
    sims = np.concatenate([r["sims"] for r in res.results], axis=0)
    mask = np.concatenate([r["mask"] for r in res.results], axis=0)
    total = sum(float(r["partial"][0, 0]) for r in res.results)
    loss = np.float32(total / B)
    if _trace:
        kernel._last_results = res
    return loss, loss, mask, sims


# revision 16
# speedup vs baseline: 3.2690x; 3.2690x over previous
"""Trainium2 Bass kernel for nn_ComplementaryContrastiveLearning.

Reference computation (B=8192, D=512, fp32):
    noisy        = confident_noisy_mask != 0
    negative_mask[i,j] = noisy[i] | noisy[j]                       (as f32)
    vn, tn       = l2_normalize(visual), l2_normalize(textual)
    sims         = vn @ tn.T                                       [B, B]
    loss         = mean(log(sum(exp(sims/T) * negative_mask, 1) + EPS))
    returns (loss, loss, negative_mask, sims)

Sharding: rows of visual (and of sims / negative_mask) split across 8 cores,
textual replicated.  Each core:
  - normalizes its visual rows and all textual rows on-device
    (inv_norm = exp(-0.5 * ln(sum(x^2))) so the scalar engine only ever needs
    the natural_log_exp_and_others table set - no table-switch thrash),
  - transposes both to feature-major via PE transposes,
  - computes the [1024, 8192] sims block on the PE (PSUM accumulation over 4
    K=128 chunks), copies PSUM -> SBUF -> DRAM,
  - folds the negative mask into the logits with one extra K=1 matmul:
    PSUM += -b_i*b_j with b_i = sqrt(BIG*T)*(1-noisy_i), so
    exp(PSUM/T) = exp(sims/T - BIG*(1-n_i)(1-n_j)): masked-out entries get
    a factor e^-30 ~ 1e-13 (negligible vs row sums ~1e3..1e6),
  - one ACT pass computes exp(PSUM/T) with a fused row-sum (accum_out),
  - log(rowsum + EPS), per-core partial sum of logs via a ones-matmul
    partition reduction; host sums partials / B.
The mask output is built from a host-broadcast [128, B] noisy row with a
per-partition tensor_scalar max, one row-block at a time.
"""

import numpy as np
from contextlib import ExitStack

import concourse.bass as bass
import concourse.tile as tile
from concourse import mybir
from concourse.bass_utils import run_bass_kernel_spmd

F32 = mybir.dt.float32

B = 8192
D = 512
N_CORES = 8
ROWS = B // N_CORES          # 1024 visual rows per core
P = 128                      # partitions
TEMP = 0.07
EPS = 1e-8
BIG = 30.0
BV = float(np.sqrt(np.float64(BIG) * np.float64(TEMP)))  # b_i scale

# matmul input interpretation: float32r = 1 cyc/row (fp22 truncated),
# float32 = 4 cyc/row (exact fp32). Flip here to trade accuracy vs speed.
MM_DT = mybir.dt.float32r


def _build(rows=ROWS, b_total=B, mm_dt=MM_DT, n_tile=512, legalize=True):
    """Build the per-core Bass program (SPMD; every core runs the same code
    on its own row slice)."""
    m_blocks = rows // P              # row blocks of 128
    n_slices = b_total // n_tile      # column slices of n_tile
    c_chunks = D // P                 # 4 contraction chunks of 128

    nc = bass.Bass("TRN2", target_bir_lowering=False, debug=False)

    visual = nc.dram_tensor("visual", [rows, D], F32, kind="ExternalInput").ap()
    textual = nc.dram_tensor("textual", [b_total, D], F32, kind="ExternalInput").ap()
    bnoisy = nc.dram_tensor("bnoisy", [P, b_total], F32, kind="ExternalInput").ap()
    noisyv = nc.dram_tensor("noisyv", [P, m_blocks], F32, kind="ExternalInput").ap()
    negbv = nc.dram_tensor("negbv", [1, rows], mm_dt, kind="ExternalInput").ap()
    bt = nc.dram_tensor("bt", [1, b_total], mm_dt, kind="ExternalInput").ap()

    sims = nc.dram_tensor("sims", [rows, b_total], F32, kind="ExternalOutput").ap()
    mask = nc.dram_tensor("mask", [rows, b_total], F32, kind="ExternalOutput").ap()
    partial = nc.dram_tensor("partial", [1, 1], F32, kind="ExternalOutput").ap()

    ident_h = nc.inline_tensor(np.eye(P, dtype=np.float32), name="ident")

    AF = mybir.ActivationFunctionType
    ALU = mybir.AluOpType
    AX = mybir.AxisListType

    with tile.TileContext(nc) as tc:
        with ExitStack() as ctx:
            const_pool = ctx.enter_context(tc.tile_pool(name="const", bufs=1))
            small_pool = ctx.enter_context(tc.tile_pool(name="small", bufs=4))
            prep_pool = ctx.enter_context(tc.tile_pool(name="prep", bufs=4))
            tT_pool = ctx.enter_context(tc.tile_pool(name="tT", bufs=2))
            sims_pool = ctx.enter_context(tc.tile_pool(name="simsb", bufs=4))
            mask_pool = ctx.enter_context(tc.tile_pool(name="maskb", bufs=2))
            stat_pool = ctx.enter_context(tc.tile_pool(name="stats", bufs=1))
            scr_pool = ctx.enter_context(tc.tile_pool(name="scr", bufs=2))
            ps_acc = ctx.enter_context(tc.tile_pool(name="ps", bufs=3, space="PSUM"))
            ps_tr = ctx.enter_context(tc.tile_pool(name="pst", bufs=2, space="PSUM"))

            # ---- constants / persistent tiles ----
            ident_t = const_pool.tile([P, P], F32, tag="ident")
            nc.sync.dma_start(ident_t[:], ident_h.ap()[:, :])
            ones_t = const_pool.tile([P, 1], F32, tag="ones")
            nc.vector.memset(ones_t[:], 1.0)
            eps_t = const_pool.tile([P, 1], F32, tag="eps")
            nc.vector.memset(eps_t[:], EPS)
            bnoisy_t = const_pool.tile([P, b_total], F32, tag="bnoisy")
            nc.sync.dma_start(bnoisy_t[:], bnoisy[:, :])
            noisyv_t = const_pool.tile([P, m_blocks], F32, tag="noisyv")
            nc.sync.dma_start(noisyv_t[:], noisyv[:, :])
            negbv_t = const_pool.tile([1, rows], mm_dt, tag="negbv")
            nc.sync.dma_start(negbv_t[:], negbv[:, :])
            bt_t = const_pool.tile([1, b_total], mm_dt, tag="bt")
            nc.sync.dma_start(bt_t[:], bt[:, :])

            # Absorb the const-DMA completions into the DVE engine clock up
            # front: walrus allows only a few sync-wait commands per
            # instruction, and the first consumer of several const tiles
            # would otherwise collect one wait per DMA lane.
            join_t = small_pool.tile([1, 1], F32, tag="join")
            nc.vector.tensor_copy(join_t[:], bnoisy_t[0:1, 0:1])
            nc.vector.tensor_copy(join_t[:], noisyv_t[0:1, 0:1])

            # visualT, normalized, feature-major: [feat_in_chunk, chunk, row]
            vT_t = const_pool.tile([P, c_chunks, rows], mm_dt, tag="vT")
            # per-row-block running stats: stats[m][:, n] = masked rowsum part
            n_units = b_total // (2 * n_tile)
            stats = [
                stat_pool.tile([P, n_units], F32, tag=f"st{m}", name=f"st{m}")
                for m in range(m_blocks)
            ]
            losscol = stat_pool.tile([P, m_blocks], F32, tag="losscol")

            def normalize_tile(src_ap, tag_prefix):
                """Load [128, D] rows, return SBUF tile of l2-normalized rows."""
                raw = prep_pool.tile([P, D], F32, tag=f"{tag_prefix}raw")
                nc.sync.dma_start(raw[:], src_ap)
                sq = scr_pool.tile([P, D], F32, tag="scr")
                ss = small_pool.tile([P, 1], F32, tag="ss")
                nc.scalar.activation(sq[:], raw[:], AF.Square, accum_out=ss[:])
                lns = small_pool.tile([P, 1], F32, tag="lns")
                nc.scalar.activation(lns[:], ss[:], AF.Ln)
                inv = small_pool.tile([P, 1], F32, tag="inv")
                nc.scalar.activation(inv[:], lns[:], AF.Exp, scale=-0.5)
                norm = prep_pool.tile([P, D], F32, tag=f"{tag_prefix}norm")
                nc.vector.tensor_scalar_mul(norm[:], raw[:], inv[:])
                return norm

            def transpose_to(dst_ap, norm_tile):
                """PE-transpose [128, D] -> [feat, chunk, 128 rows] into dst."""
                pst = ps_tr.tile([P, c_chunks, P], F32, tag="pst")
                for c in range(c_chunks):
                    nc.tensor.transpose(
                        pst[:, c, :], norm_tile[:, c * P:(c + 1) * P], ident_t[:]
                    )
                nc.vector.tensor_copy(dst_ap, pst[:])

            # ---- prologue: normalize + transpose the visual chunk ----
            for m in range(m_blocks):
                vn = normalize_tile(visual[m * P:(m + 1) * P, :], "v")
                transpose_to(vT_t[:, :, m * P:(m + 1) * P], vn)

            # ---- negative-mask output (independent of the matmul path) ----
            mchunk = min(2048, b_total)
            for m in range(m_blocks):
                for j in range(0, b_total, mchunk):
                    mb_t = mask_pool.tile([P, mchunk], F32, tag="maskblk")
                    nc.vector.tensor_scalar(
                        out=mb_t[:],
                        in0=bnoisy_t[:, j:j + mchunk],
                        scalar1=noisyv_t[:, m:m + 1],
                        scalar2=None,
                        op0=ALU.max,
                    )
                    nc.sync.dma_start(
                        mask[m * P:(m + 1) * P, j:j + mchunk], mb_t[:])

            # ---- main loop: textual prep + matmul + (delayed) fold/exp ----
            # Each unit is a [128, 2*n_tile] PSUM accumulator (2 banks).  The
            # mask-fold K=1 matmul + exp of unit u-1 are emitted after unit
            # u's main matmuls so the PE never stalls waiting on the sims
            # copy of the unit it just produced (keeps PE dense -> HAM warm).
            unit = 2 * n_tile
            pending = None

            def fold_and_exp(ps, m, u):
                for h in range(2):
                    j0 = u * unit + h * n_tile
                    nc.tensor.matmul(
                        ps[:, h * n_tile:(h + 1) * n_tile],
                        lhsT=negbv_t[0:1, m * P:(m + 1) * P],
                        rhs=bt_t[0:1, j0:j0 + n_tile],
                        start=False,
                        stop=True,
                        skip_group_check=True,
                    )
                nc.scalar.activation(
                    ps[:], ps[:], AF.Exp,
                    scale=1.0 / TEMP,
                    accum_out=stats[m][:, u:u + 1],
                )

            for u in range(n_units):
                tTn = tT_pool.tile([P, c_chunks, unit], mm_dt, tag="tTn")
                for t in range(unit // P):
                    row0 = u * unit + t * P
                    tn_tile = normalize_tile(textual[row0:row0 + P, :], "t")
                    transpose_to(tTn[:, :, t * P:(t + 1) * P], tn_tile)

                for m in range(m_blocks):
                    ps = ps_acc.tile([P, unit], F32, tag="acc")
                    for h in range(2):
                        for c in range(c_chunks):
                            nc.tensor.matmul(
                                ps[:, h * n_tile:(h + 1) * n_tile],
                                lhsT=vT_t[:, c, m * P:(m + 1) * P],
                                rhs=tTn[:, c, h * n_tile:(h + 1) * n_tile],
                                start=(c == 0),
                                stop=(c == c_chunks - 1),
                            )
                    stile = sims_pool.tile([P, unit], F32, tag="stile")
                    nc.vector.tensor_copy(stile[:], ps[:])
                    nc.sync.dma_start(
                        sims[m * P:(m + 1) * P, u * unit:(u + 1) * unit],
                        stile[:],
                    )
                    if pending is not None:
                        fold_and_exp(*pending)
                    pending = (ps, m, u)
            fold_and_exp(*pending)

            # ---- epilogue: log(rowsum + eps), partition-sum, partial out ----
            for m in range(m_blocks):
                rowsum = small_pool.tile([P, 1], F32, tag="rowsum")
                nc.vector.reduce_sum(rowsum[:], stats[m][:], axis=AX.X)
                nc.scalar.activation(
                    losscol[:, m:m + 1], rowsum[:], AF.Ln, bias=eps_t[:]
                )
            total = small_pool.tile([P, 1], F32, tag="total")
            nc.vector.reduce_sum(total[:], losscol[:], axis=AX.X)
            ps1 = ps_acc.tile([1, 1], F32, tag="acc")
            nc.tensor.matmul(ps1[:], lhsT=ones_t[:], rhs=total[:],
                             start=True, stop=True)
            out1 = small_pool.tile([1, 1], F32, tag="out1")
            nc.vector.tensor_copy(out1[:], ps1[:])
            nc.sync.dma_start(partial[:, :], out1[:])

    return _legalize_waits(nc) if legalize else nc


def _legalize_waits(nc):
    """This toolchain's walrus codegen accepts at most ONE sync-wait command
    per instruction; the Tile scheduler freely attaches several.  Hoist the
    extras onto standalone EventSemaphore instructions (same engine, directly
    before the instruction) - program order on the engine preserves the gate
    semantics exactly."""
    for func in nc.m.functions:
        for block in func.blocks:
            out = []
            for ins in block.instructions:
                si = ins.sync_info
                waits = list(si.on_wait) if si is not None and si.on_wait else []
                if len(waits) > 1:
                    for k, w in enumerate(waits[:-1]):
                        ev = mybir.InstEventSemaphore(name=f"{ins.name}-ew{k}")
                        ev.engine = ins.engine
                        ev.sync_info = mybir.SyncInfo(on_wait=[w], on_update=[])
                        out.append(ev)
                    ins.sync_info = mybir.SyncInfo(
                        on_wait=[waits[-1]],
                        on_update=list(si.on_update) if si.on_update else [],
                    )
                out.append(ins)
            if len(out) != len(block.instructions):
                block.instructions = out
    return nc


_NC_CACHE = {}


def _get_nc():
    key = (ROWS, B, MM_DT)
    if key not in _NC_CACHE:
        _NC_CACHE[key] = _build()
    return _NC_CACHE[key]


def _host_inputs(visual_feats, textual_feats, confident_noisy_mask):
    vis = np.ascontiguousarray(np.asarray(visual_feats, dtype=np.float32))
    tex = np.ascontiguousarray(np.asarray(textual_feats, dtype=np.float32))
    noisy_f = (np.asarray(confident_noisy_mask) != 0).astype(np.float32)
    b_vec = (BV * (1.0 - noisy_f)).astype(np.float32)          # [B]
    bnoisy = np.ascontiguousarray(np.broadcast_to(noisy_f, (P, B)))
    bt_2d = np.ascontiguousarray(b_vec.reshape(1, B))
    m_blocks = ROWS // P
    in_maps = []
    for c in range(N_CORES):
        rs = slice(c * ROWS, (c + 1) * ROWS)
        in_maps.append({
            "visual": vis[rs],
            "textual": tex,
            "bnoisy": bnoisy,
            "noisyv": np.ascontiguousarray(
                noisy_f[rs].reshape(m_blocks, P).T),
            "negbv": np.ascontiguousarray((-b_vec[rs]).reshape(1, ROWS)),
            "bt": bt_2d,
        })
    return in_maps


def kernel(visual_feats, textual_feats, confident_clean_mask,
           confident_noisy_mask, _trace=False):
    nc = _get_nc()
    in_maps = _host_inputs(visual_feats, textual_feats, confident_noisy_mask)
    res = run_bass_kernel_spmd(nc, in_maps, list(range(N_CORES)), trace=_trace)
    sims = np.concatenate([r["sims"] for r in res.results], axis=0)
    mask = np.concatenate([r["mask"] for r in res.results], axis=0)
    total = sum(float(r["partial"][0, 0]) for r in res.results)
    loss = np.float32(total / B)
    if _trace:
        kernel._last_results = res
    return loss, loss, mask, sims


# revision 18
# speedup vs baseline: 3.4536x; 1.0565x over previous
"""Trainium2 Bass kernel for nn_ComplementaryContrastiveLearning.

Reference computation (B=8192, D=512, fp32):
    noisy        = confident_noisy_mask != 0
    negative_mask[i,j] = noisy[i] | noisy[j]                       (as f32)
    vn, tn       = l2_normalize(visual), l2_normalize(textual)
    sims         = vn @ tn.T                                       [B, B]
    loss         = mean(log(sum(exp(sims/T) * negative_mask, 1) + EPS))
    returns (loss, loss, negative_mask, sims)

Sharding: rows of visual (and of sims / negative_mask) split across 8 cores,
textual replicated.  Each core:
  - normalizes its visual rows and all textual rows on-device
    (inv_norm = exp(-0.5 * ln(sum(x^2))) so the scalar engine only ever needs
    the natural_log_exp_and_others table set - no table-switch thrash),
  - transposes both to feature-major via PE transposes,
  - computes the [1024, 8192] sims block on the PE (PSUM accumulation over 4
    K=128 chunks), copies PSUM -> SBUF -> DRAM,
  - folds the negative mask into the logits with one extra K=1 matmul:
    PSUM += -b_i*b_j with b_i = sqrt(BIG*T)*(1-noisy_i), so
    exp(PSUM/T) = exp(sims/T - BIG*(1-n_i)(1-n_j)): masked-out entries get
    a factor e^-30 ~ 1e-13 (negligible vs row sums ~1e3..1e6),
  - one ACT pass computes exp(PSUM/T) with a fused row-sum (accum_out),
  - log(rowsum + EPS), per-core partial sum of logs via a ones-matmul
    partition reduction; host sums partials / B.
The mask output is built from a host-broadcast [128, B] noisy row with a
per-partition tensor_scalar max, one row-block at a time.
"""

import numpy as np
from contextlib import ExitStack

import concourse.bass as bass
import concourse.tile as tile
from concourse import mybir
from concourse.bass_utils import run_bass_kernel_spmd

F32 = mybir.dt.float32

B = 8192
D = 512
N_CORES = 8
ROWS = B // N_CORES          # 1024 visual rows per core
P = 128                      # partitions
TEMP = 0.07
EPS = 1e-8
BIG = 30.0
BV = float(np.sqrt(np.float64(BIG) * np.float64(TEMP)))  # b_i scale

# matmul input interpretation: float32r = 1 cyc/row (fp22 truncated),
# float32 = 4 cyc/row (exact fp32). Flip here to trade accuracy vs speed.
MM_DT = mybir.dt.float32r


def _build(rows=ROWS, b_total=B, mm_dt=MM_DT, n_tile=512, legalize=True):
    """Build the per-core Bass program (SPMD; every core runs the same code
    on its own row slice)."""
    m_blocks = rows // P              # row blocks of 128
    n_slices = b_total // n_tile      # column slices of n_tile
    c_chunks = D // P                 # 4 contraction chunks of 128

    nc = bass.Bass("TRN2", target_bir_lowering=False, debug=False)

    visual = nc.dram_tensor("visual", [rows, D], F32, kind="ExternalInput").ap()
    textual = nc.dram_tensor("textual", [b_total, D], F32, kind="ExternalInput").ap()
    bnoisy = nc.dram_tensor("bnoisy", [P, b_total], F32, kind="ExternalInput").ap()
    noisyv = nc.dram_tensor("noisyv", [P, m_blocks], F32, kind="ExternalInput").ap()
    negbv = nc.dram_tensor("negbv", [1, rows], mm_dt, kind="ExternalInput").ap()
    bt = nc.dram_tensor("bt", [1, b_total], mm_dt, kind="ExternalInput").ap()

    sims = nc.dram_tensor("sims", [rows, b_total], F32, kind="ExternalOutput").ap()
    mask = nc.dram_tensor("mask", [rows, b_total], F32, kind="ExternalOutput").ap()
    partial = nc.dram_tensor("partial", [1, 1], F32, kind="ExternalOutput").ap()

    ident_h = nc.inline_tensor(np.eye(P, dtype=np.float32), name="ident")

    AF = mybir.ActivationFunctionType
    ALU = mybir.AluOpType
    AX = mybir.AxisListType

    with tile.TileContext(nc) as tc:
        with ExitStack() as ctx:
            const_pool = ctx.enter_context(tc.tile_pool(name="const", bufs=1))
            small_pool = ctx.enter_context(tc.tile_pool(name="small", bufs=4))
            prep_pool = ctx.enter_context(tc.tile_pool(name="prep", bufs=4))
            tT_pool = ctx.enter_context(tc.tile_pool(name="tT", bufs=2))
            sims_pool = ctx.enter_context(tc.tile_pool(name="simsb", bufs=4))
            mask_pool = ctx.enter_context(tc.tile_pool(name="maskb", bufs=2))
            stat_pool = ctx.enter_context(tc.tile_pool(name="stats", bufs=1))
            scr_pool = ctx.enter_context(tc.tile_pool(name="scr", bufs=2))
            ps_acc = ctx.enter_context(tc.tile_pool(name="ps", bufs=3, space="PSUM"))
            ps_tr = ctx.enter_context(tc.tile_pool(name="pst", bufs=2, space="PSUM"))

            # ---- constants / persistent tiles ----
            ident_t = const_pool.tile([P, P], F32, tag="ident")
            nc.sync.dma_start(ident_t[:], ident_h.ap()[:, :])
            ones_t = const_pool.tile([P, 1], F32, tag="ones")
            nc.vector.memset(ones_t[:], 1.0)
            eps_t = const_pool.tile([P, 1], F32, tag="eps")
            nc.vector.memset(eps_t[:], EPS)
            bnoisy_t = const_pool.tile([P, b_total], F32, tag="bnoisy")
            nc.scalar.dma_start(bnoisy_t[:], bnoisy[:, :])
            noisyv_t = const_pool.tile([P, m_blocks], F32, tag="noisyv")
            nc.sync.dma_start(noisyv_t[:], noisyv[:, :])
            negbv_t = const_pool.tile([1, rows], mm_dt, tag="negbv")
            nc.sync.dma_start(negbv_t[:], negbv[:, :])
            bt_t = const_pool.tile([1, b_total], mm_dt, tag="bt")
            nc.sync.dma_start(bt_t[:], bt[:, :])

            # Absorb the const-DMA completions into the DVE engine clock up
            # front: walrus allows only a few sync-wait commands per
            # instruction, and the first consumer of several const tiles
            # would otherwise collect one wait per DMA lane.
            join_t = small_pool.tile([1, 1], F32, tag="join")
            nc.vector.tensor_copy(join_t[:], bnoisy_t[0:1, 0:1])
            nc.vector.tensor_copy(join_t[:], noisyv_t[0:1, 0:1])

            # visualT, normalized, feature-major: [feat_in_chunk, chunk, row]
            vT_t = const_pool.tile([P, c_chunks, rows], mm_dt, tag="vT")
            # per-row-block running stats: stats[m][:, n] = masked rowsum part
            n_units = b_total // (2 * n_tile)
            stats = [
                stat_pool.tile([P, n_units], F32, tag=f"st{m}", name=f"st{m}")
                for m in range(m_blocks)
            ]
            losscol = stat_pool.tile([P, m_blocks], F32, tag="losscol")

            def normalize_tile(src_ap, tag_prefix):
                """Load [128, D] rows, return SBUF tile of l2-normalized rows."""
                raw = prep_pool.tile([P, D], F32, tag=f"{tag_prefix}raw")
                nc.sync.dma_start(raw[:], src_ap)
                sq = scr_pool.tile([P, D], F32, tag="scr")
                ss = small_pool.tile([P, 1], F32, tag="ss")
                nc.scalar.activation(sq[:], raw[:], AF.Square, accum_out=ss[:])
                lns = small_pool.tile([P, 1], F32, tag="lns")
                nc.scalar.activation(lns[:], ss[:], AF.Ln)
                inv = small_pool.tile([P, 1], F32, tag="inv")
                nc.scalar.activation(inv[:], lns[:], AF.Exp, scale=-0.5)
                norm = prep_pool.tile([P, D], F32, tag=f"{tag_prefix}norm")
                nc.vector.tensor_scalar_mul(norm[:], raw[:], inv[:])
                return norm

            def transpose_to(dst_ap, norm_tile):
                """PE-transpose [128, D] -> [feat, chunk, 128 rows] into dst."""
                pst = ps_tr.tile([P, c_chunks, P], F32, tag="pst")
                for c in range(c_chunks):
                    nc.tensor.transpose(
                        pst[:, c, :], norm_tile[:, c * P:(c + 1) * P], ident_t[:]
                    )
                nc.vector.tensor_copy(dst_ap, pst[:])

            # ---- prologue: normalize + transpose the visual chunk ----
            for m in range(m_blocks):
                vn = normalize_tile(visual[m * P:(m + 1) * P, :], "v")
                transpose_to(vT_t[:, :, m * P:(m + 1) * P], vn)

            # negative-mask output: emitted interleaved into the unit loop (one
            # row-block per unit) so its 32 MB of stores never monopolize the
            # DMA rings ahead of the textual loads.
            mchunk = min(2048, b_total)

            def emit_mask_block(m):
                for j in range(0, b_total, mchunk):
                    mb_t = mask_pool.tile([P, mchunk], F32, tag="maskblk")
                    nc.vector.tensor_scalar(
                        out=mb_t[:],
                        in0=bnoisy_t[:, j:j + mchunk],
                        scalar1=noisyv_t[:, m:m + 1],
                        scalar2=None,
                        op0=ALU.max,
                    )
                    nc.gpsimd.dma_start(
                        mask[m * P:(m + 1) * P, j:j + mchunk], mb_t[:])

            # ---- main loop: textual prep + matmul + (delayed) fold/exp ----
            # Each unit is a [128, 2*n_tile] PSUM accumulator (2 banks).  The
            # mask-fold K=1 matmul + exp of unit u-1 are emitted after unit
            # u's main matmuls so the PE never stalls waiting on the sims
            # copy of the unit it just produced (keeps PE dense -> HAM warm).
            unit = 2 * n_tile
            pending = None

            def fold_and_exp(ps, m, u):
                for h in range(2):
                    j0 = u * unit + h * n_tile
                    nc.tensor.matmul(
                        ps[:, h * n_tile:(h + 1) * n_tile],
                        lhsT=negbv_t[0:1, m * P:(m + 1) * P],
                        rhs=bt_t[0:1, j0:j0 + n_tile],
                        start=False,
                        stop=True,
                        skip_group_check=True,
                    )
                nc.scalar.activation(
                    ps[:], ps[:], AF.Exp,
                    scale=1.0 / TEMP,
                    accum_out=stats[m][:, u:u + 1],
                )

            for u in range(n_units):
                tTn = tT_pool.tile([P, c_chunks, unit], mm_dt, tag="tTn")
                for t in range(unit // P):
                    row0 = u * unit + t * P
                    tn_tile = normalize_tile(textual[row0:row0 + P, :], "t")
                    transpose_to(tTn[:, :, t * P:(t + 1) * P], tn_tile)
                for mm_ in range(u * m_blocks // n_units,
                                 (u + 1) * m_blocks // n_units):
                    emit_mask_block(mm_)

                for m in range(m_blocks):
                    ps = ps_acc.tile([P, unit], F32, tag="acc")
                    for h in range(2):
                        for c in range(c_chunks):
                            nc.tensor.matmul(
                                ps[:, h * n_tile:(h + 1) * n_tile],
                                lhsT=vT_t[:, c, m * P:(m + 1) * P],
                                rhs=tTn[:, c, h * n_tile:(h + 1) * n_tile],
                                start=(c == 0),
                                stop=(c == c_chunks - 1),
                            )
                    stile = sims_pool.tile([P, unit], F32, tag="stile")
                    nc.vector.tensor_copy(stile[:], ps[:])
                    store_eng = nc.sync if (m % 2 == 0) else nc.scalar
                    store_eng.dma_start(
                        sims[m * P:(m + 1) * P, u * unit:(u + 1) * unit],
                        stile[:],
                    )
                    if pending is not None:
                        fold_and_exp(*pending)
                    pending = (ps, m, u)
            fold_and_exp(*pending)

            # ---- epilogue: log(rowsum + eps), partition-sum, partial out ----
            for m in range(m_blocks):
                rowsum = small_pool.tile([P, 1], F32, tag="rowsum")
                nc.vector.reduce_sum(rowsum[:], stats[m][:], axis=AX.X)
                nc.scalar.activation(
                    losscol[:, m:m + 1], rowsum[:], AF.Ln, bias=eps_t[:]
                )
            total = small_pool.tile([P, 1], F32, tag="total")
            nc.vector.reduce_sum(total[:], losscol[:], axis=AX.X)
            ps1 = ps_acc.tile([1, 1], F32, tag="acc")
            nc.tensor.matmul(ps1[:], lhsT=ones_t[:], rhs=total[:],
                             start=True, stop=True)
            out1 = small_pool.tile([1, 1], F32, tag="out1")
            nc.vector.tensor_copy(out1[:], ps1[:])
            nc.sync.dma_start(partial[:, :], out1[:])

    return _legalize_waits(nc) if legalize else nc


def _legalize_waits(nc):
    """This toolchain's walrus codegen accepts at most ONE sync-wait command
    per instruction; the Tile scheduler freely attaches several.  Hoist the
    extras onto standalone EventSemaphore instructions (same engine, directly
    before the instruction) - program order on the engine preserves the gate
    semantics exactly."""
    for func in nc.m.functions:
        for block in func.blocks:
            out = []
            for ins in block.instructions:
                si = ins.sync_info
                waits = list(si.on_wait) if si is not None and si.on_wait else []
                if len(waits) > 1:
                    for k, w in enumerate(waits[:-1]):
                        ev = mybir.InstEventSemaphore(name=f"{ins.name}-ew{k}")
                        ev.engine = ins.engine
                        ev.sync_info = mybir.SyncInfo(on_wait=[w], on_update=[])
                        out.append(ev)
                    ins.sync_info = mybir.SyncInfo(
                        on_wait=[waits[-1]],
                        on_update=list(si.on_update) if si.on_update else [],
                    )
                out.append(ins)
            if len(out) != len(block.instructions):
                block.instructions = out
    return nc


_NC_CACHE = {}


def _get_nc():
    key = (ROWS, B, MM_DT)
    if key not in _NC_CACHE:
        _NC_CACHE[key] = _build()
    return _NC_CACHE[key]


def _host_inputs(visual_feats, textual_feats, confident_noisy_mask):
    vis = np.ascontiguousarray(np.asarray(visual_feats, dtype=np.float32))
    tex = np.ascontiguousarray(np.asarray(textual_feats, dtype=np.float32))
    noisy_f = (np.asarray(confident_noisy_mask) != 0).astype(np.float32)
    b_vec = (BV * (1.0 - noisy_f)).astype(np.float32)          # [B]
    bnoisy = np.ascontiguousarray(np.broadcast_to(noisy_f, (P, B)))
    bt_2d = np.ascontiguousarray(b_vec.reshape(1, B))
    m_blocks = ROWS // P
    in_maps = []
    for c in range(N_CORES):
        rs = slice(c * ROWS, (c + 1) * ROWS)
        in_maps.append({
            "visual": vis[rs],
            "textual": tex,
            "bnoisy": bnoisy,
            "noisyv": np.ascontiguousarray(
                noisy_f[rs].reshape(m_blocks, P).T),
            "negbv": np.ascontiguousarray((-b_vec[rs]).reshape(1, ROWS)),
            "bt": bt_2d,
        })
    return in_maps


def kernel(visual_feats, textual_feats, confident_clean_mask,
           confident_noisy_mask, _trace=False):
    nc = _get_nc()
    in_maps = _host_inputs(visual_feats, textual_feats, confident_noisy_mask)
    res = run_bass_kernel_spmd(nc, in_maps, list(range(N_CORES)), trace=_trace)
    sims = np.concatenate([r["sims"] for r in res.results], axis=0)
    mask = np.concatenate([r["mask"] for r in res.results], axis=0)
    total = sum(float(r["partial"][0, 0]) for r in res.results)
    loss = np.float32(total / B)
    if _trace:
        kernel._last_results = res
    return loss, loss, mask, sims
